# revision 1
# baseline (speedup 1.0000x reference)
"""Trainium2 Bass kernel for nn_CapsuleLayer (B=32, In=128, Din=256, ch=32, Nc=47, Dc=64).

Sharding: over the OUTPUT-CAPSULE axis Nc (47 -> pad 48 = 8 cores x 6 capsules).
Routing is fully independent per (batch, output-capsule), and W (94 MiB) is the
dominant HBM tensor -- Nc-sharding reads W exactly once total (12.6 MiB/core)
instead of replicating it 8x as batch-sharding would.

Per-core layout ("layout R"): inputs_hat stored [p=(b,rr) 128 partitions,
(c, n, k) 12288 free] -- produced directly by per-channel matmuls
ih_c[(b,rr),(n,k)] = sum_d xT_c[d,(b,rr)]^T . WT_c[d,(n,k)].
Partition-group (rr / b-group) reductions run on the PE with block-diagonal
ones matrices (BD4 [128,32], BD4T [32,128]); free-dim (c, k) reductions on the
DVE.  Routing iteration t:
  a  = sum_k OUT_{t-1} * ih        (DVE mul + reduce-X over k)
  E  = exp(sum_t a)                (ACT; softmax normalization folded into s)
  s  = (sum_i E*ih)/Z + B          (DVE mul + reduce-X over c, PE BD4 matmul)
  OUT = squash(s)                  (small [32,384] ops)
Iteration 1 (uniform c) is s1 = BD4^T IH summed over channels on the PE.

Toolchain constraint: matmul (S3_LW) and DMA (DIRECT2D) instructions accept at
most ONE sync wait at codegen.  Hence: the x|w stream is 8 one-shot DMAs into
disjoint regions of one resident SBUF tile (no slot WAR/WAW, one queue each);
both matmul operands come from the same stream region (one DMA sem); const
DMAs ride SWDGE and are pre-absorbed into the PE clock via tiny dummy matmuls;
psum-slot WAR ticks are absorbed the same way.  The routing scratch TMP
aliases the stream tile (dead after phase 1).
"""

import numpy as np

B, IN, DIN = 32, 128, 256
CH, NC, DC = 32, 47, 64
NCP = 48          # padded Nc
NSH = 6           # capsules per core
NCORES = 8
NK = NSH * DC     # 384
EPS = 1e-7

_cache = {}


def _build_nc():
    import concourse.bass as bass
    import concourse.tile as tile
    from concourse import mybir
    from concourse.tile_rust import add_dep_helper

    f32 = mybir.dt.float32
    nc = bass.Bass()

    # packed stream: per cd-chunk rows [xt(128) | wt(384)] = 512 floats
    xw = nc.dram_tensor("xw", [CH * 2, 128, 512], f32, kind="ExternalInput")
    # consts packed in one tensor: [bd4(0:32) | bd4t(rows0:32, 32:160) |
    # brep(rows0:32, 160:544)]
    cst = nc.dram_tensor("cst", [128, 544], f32, kind="ExternalInput")
    out_d = nc.dram_tensor("out", [B, NK], f32, kind="ExternalOutput")

    ADD = mybir.AluOpType.add
    MULT = mybir.AluOpType.mult
    AX = mybir.AxisListType.X
    AF = mybir.ActivationFunctionType

    with tile.TileContext(nc) as tc:
        with (
            tc.tile_pool(name="singles", bufs=1) as singles,
            tc.tile_pool(name="work", bufs=1) as work,
            tc.tile_pool(name="small", bufs=2) as small,
            tc.tile_pool(name="ps_ih", bufs=2, space="PSUM") as ps_ih,
            tc.tile_pool(name="ps_s1", bufs=1, space="PSUM") as ps_s1,
            tc.tile_pool(name="ps_rep", bufs=1, space="PSUM") as ps_rep,
            tc.tile_pool(name="ps_sm", bufs=1, space="PSUM") as ps_sm,
            tc.tile_pool(name="ps_s2", bufs=2, space="PSUM") as ps_s2,
        ):
            cst_t = singles.tile([128, 544], f32)
            c_dma = nc.sync.dma_start(out=cst_t[:], in_=cst[:])
            bd4_t = cst_t[:, 0:B]
            bd4t_t = cst_t[0:B, B:B + 128]
            brep_t = cst_t[0:B, B + 128:B + 128 + NK]
            eps_t = singles.tile([B, 1], f32)
            nc.vector.memset(eps_t[:], EPS)
            # DVE is also 1-wait-limited: pre-observe the const-DMA sem
            dve_scratch = singles.tile([B, 8], f32)
            nc.vector.tensor_copy(dve_scratch[:, 0:2], cst_t[:B, :2])

            IH = singles.tile([128, CH, NK], f32)
            # resident packed stream; 8 one-shot DMAs into disjoint regions
            STREAM = singles.tile([128, CH * 2, 512], f32)
            # routing scratch aliases the stream area (dead after phase 1)
            TMP = (STREAM[:]
                   .rearrange("p a b -> p (a b)")[:, :CH * NK]
                   .rearrange("p (c nk) -> p c nk", c=CH))

            # Absorb the const-DMA sem into the PE clock (dummy matmul) so
            # real matmuls carry a single wait.
            pd = ps_sm.tile([2, 2], f32, tag="dummy")
            last_dummy = nc.tensor.matmul(
                pd[:], cst_t[:2, :2], cst_t[:2, :2], start=True, stop=True,
                skip_group_check=True,
            )

            psum_s1 = ps_s1.tile([B, NK], f32)

            # ---------------- phase 1: inputs_hat + iter-1 s ----------------
            # 2 big stream DMAs: each splits across all 16 SDMA engine slots
            # internally (full HBM BW), while keeping the kernel-tail drain's
            # semaphore count low (wait-slot limit on the drain).
            s_dmas = []
            for g in range(2):
                s_dmas.append(nc.sync.dma_start(
                    out=STREAM[:, 32 * g:32 * (g + 1), :],
                    in_=xw[32 * g:32 * (g + 1)].rearrange("q d f -> d q f"),
                ))
            # DVE pre-observes the stream-DMA sems (TMP aliases the stream)
            stream_scratch = singles.tile([128, 16], f32)
            for g in range(2):
                nc.vector.tensor_copy(stream_scratch[:, 2 * g:2 * g + 2],
                                      STREAM[:, 32 * g, :2])
            copy_insts = []
            for c in range(CH):
                if c >= 2:
                    # absorb the psum-slot WAR tick (copy of c-2, DVE/ACT sem)
                    dmy = nc.tensor.matmul(pd[:], bd4_t[:, :2], bd4_t[:, :2],
                                           start=True, stop=True,
                                           skip_group_check=True)
                    add_dep_helper(dmy.ins, copy_insts[c - 2].ins, sync=True,
                                   reason="absorb psum WAR tick on PE")
                    last_dummy = dmy
                psum_ih = ps_ih.tile([128, NK], f32, tag="ih")
                for dc in range(2):
                    cd = c * 2 + dc
                    mih = nc.tensor.matmul(
                        psum_ih[:], STREAM[:, cd, 0:128], STREAM[:, cd, 128:512],
                        start=(dc == 0), stop=(dc == 1),
                    )
                    if dc == 0:
                        add_dep_helper(mih.ins, last_dummy.ins, sync=False,
                                       reason="order dummy before matmul")
                # spread PSUM->SBUF copies across DVE and ACT
                if c % 2 == 0:
                    copy_insts.append(nc.vector.tensor_copy(IH[:, c, :], psum_ih[:]))
                else:
                    copy_insts.append(nc.scalar.copy(IH[:, c, :], psum_ih[:]))
                # iter-1 s accumulation: sum_rr over partitions via BD4, and
                # over channels via PSUM accumulation
                nc.tensor.matmul(
                    psum_s1[:], bd4_t[:], IH[:, c, :],
                    start=(c == 0), stop=(c == CH - 1),
                    skip_group_check=True,
                )

            _absn = [0]

            def absorb(eng, src_ap, target_hint=""):
                """Tiny copy on `eng` reading src_ap: pre-observes the
                producer's sem so the next real op keeps a single wait."""
                _absn[0] += 1
                scr = small.tile([2, 2], f32, tag="abs%d" % _absn[0])
                if eng == "v":
                    return nc.vector.tensor_copy(scr[:], src_ap)
                return nc.scalar.copy(scr[:], src_ap)

            def squash(S, tag):
                """S: [B, NK] sbuf tile -> OUT [B, NK] sbuf tile."""
                Ssq = work.tile([B, NK], f32, tag="Su")
                nc.vector.tensor_mul(Ssq[:], S[:], S[:])
                m2 = small.tile([B, NSH], f32, tag="m2")
                nc.vector.tensor_reduce(
                    m2[:],
                    Ssq[:].rearrange("p (n k) -> p n k", n=NSH),
                    axis=AX, op=ADD,
                )
                d1 = small.tile([B, NSH], f32, tag="d1")
                nc.vector.tensor_scalar_add(d1[:], m2[:], 1.0)
                absorb("s", m2[:2, :2])          # ACT clock <- m2 (DVE)
                sq = small.tile([B, NSH], f32, tag="sq")
                nc.scalar.activation(sq[:], m2[:], AF.Sqrt, bias=eps_t[:])
                absorb("v", sq[:2, :2])          # DVE clock <- sq (ACT)
                den = small.tile([B, NSH], f32, tag="den")
                nc.vector.tensor_mul(den[:], d1[:], sq[:])
                rden = small.tile([B, NSH], f32, tag="rden")
                nc.vector.reciprocal(rden[:], den[:])
                g_ = small.tile([B, NSH], f32, tag="g")
                nc.vector.tensor_mul(g_[:], m2[:], rden[:])
                OUT = work.tile([B, NK], f32, tag="out")
                nc.vector.tensor_mul(
                    OUT[:].rearrange("p (n k) -> p n k", n=NSH),
                    S[:].rearrange("p (n k) -> p n k", n=NSH),
                    g_[:].rearrange("p (n o) -> p n o", o=1).broadcast_to([B, NSH, DC]),
                )
                return OUT

            def replicate(OUT, tag):
                """OUT [B, NK] -> [128, NK] (row b to partitions 4b..4b+3)."""
                pr = ps_rep.tile([128, NK], f32, tag="rep")
                nc.tensor.matmul(pr[:], bd4t_t[:], OUT[:], start=True, stop=True)
                R = work.tile([128, NK], f32, tag="rep")
                nc.vector.tensor_copy(R[:], pr[:])
                return R

            # ---------------- iter 1 ----------------
            S1 = work.tile([B, NK], f32, tag="S")
            nc.vector.scalar_tensor_tensor(
                out=S1[:], in0=psum_s1[:], scalar=1.0 / IN, in1=brep_t[:],
                op0=MULT, op1=ADD,
            )
            OUT1 = squash(S1, "1")
            OUTr = replicate(OUT1, "1")

            Aprev = None
            for it in (2, 3):
                # a-step: A = sum_k OUTr * IH   -> [128, CH, NSH]
                absorb("v", IH[:2, CH - 1, :2])   # DVE clock <- last ACT copy
                nc.vector.tensor_mul(
                    TMP,
                    IH[:],
                    OUTr[:].rearrange("p (o nk) -> p o nk", o=1)
                          .broadcast_to([128, CH, NK]),
                )
                A = work.tile([128, CH, NSH], f32, tag="A%d" % it)
                nc.vector.tensor_reduce(
                    A[:],
                    TMP.rearrange("p c (n k) -> p c n k", n=NSH),
                    axis=AX, op=ADD,
                )
                if Aprev is None:
                    BL = A
                    Aprev = A
                else:
                    BL = work.tile([128, CH, NSH], f32, tag="BL")
                    nc.vector.tensor_add(BL[:], A[:], Aprev[:])
                # E = exp(BL)
                absorb("s", BL[:2, 0, :2])        # ACT clock <- BL (DVE)
                E = work.tile([128, CH, NSH], f32, tag="E")
                nc.scalar.activation(E[:], BL[:], AF.Exp)
                # Zp[p, n] = sum_c E ; Z = BD4 reduction over rr
                absorb("v", E[:2, 0, :2])         # DVE clock <- E (ACT)
                Zp = small.tile([128, NSH], f32, tag="Zp")
                nc.vector.tensor_reduce(
                    Zp[:],
                    E[:].rearrange("p c n -> p n c"),
                    axis=AX, op=ADD,
                )
                pz = ps_sm.tile([B, NSH], f32, tag="pz")
                nc.tensor.matmul(pz[:], bd4_t[:], Zp[:], start=True, stop=True)
                # s-step: TMP = E*IH ; P2 = sum_c ; S = BD4(P2)/Z + brep
                nc.vector.tensor_mul(
                    TMP.rearrange("p c (n k) -> p c n k", n=NSH),
                    IH[:].rearrange("p c (n k) -> p c n k", n=NSH),
                    E[:].rearrange("p c (n o) -> p c n o", o=1)
                       .broadcast_to([128, CH, NSH, DC]),
                )
                P2 = work.tile([128, NK], f32, tag="P2")
                nc.vector.tensor_reduce(
                    P2[:].rearrange("p (n k) -> p n k", n=NSH),
                    TMP.rearrange("p c (n k) -> p n k c", n=NSH),
                    axis=AX, op=ADD,
                )
                pS = ps_s2.tile([B, NK], f32, tag="pS")
                mm_last = nc.tensor.matmul(pS[:], bd4_t[:], P2[:],
                                           start=True, stop=True)
                Zs = small.tile([B, NSH], f32, tag="Zs")
                nc.vector.tensor_copy(Zs[:], pz[:])
                Rz = small.tile([B, NSH], f32, tag="Rz")
                nc.vector.reciprocal(Rz[:], Zs[:])
                absorb("v", pS[:2, :2])          # DVE clock <- pS (PE)
                Su = work.tile([B, NK], f32, tag="Su")
                nc.vector.tensor_mul(
                    Su[:].rearrange("p (n k) -> p n k", n=NSH),
                    pS[:].rearrange("p (n k) -> p n k", n=NSH),
                    Rz[:].rearrange("p (n o) -> p n o", o=1).broadcast_to([B, NSH, DC]),
                )
                S = work.tile([B, NK], f32, tag="S")
                nc.vector.tensor_add(S[:], Su[:], brep_t[:])
                OUT = squash(S, str(it))
                if it < 3:
                    OUTr = replicate(OUT, str(it))
                else:
                    o_dma = nc.sync.dma_start(out=out_d[:], in_=OUT[:])
                    # Pre-absorb every final sem into the SYNC engine so the
                    # Tile kernel-tail drain needs <=1 wait (codegen limit).
                    f_scr = small.tile([2, 4], f32, tag="fin")
                    f_act = nc.scalar.copy(f_scr[:, 0:2], OUT[:2, :2])
                    f_dve = nc.vector.tensor_copy(f_scr[:, 2:4], OUT[:2, :2])
                    for fin in (c_dma, s_dmas[0], s_dmas[1], mm_last,
                                f_act, f_dve, o_dma):
                        fnop = nc.sync.nop()
                        add_dep_helper(fnop.ins, fin.ins, sync=True,
                                       reason="absorb final sem for tail drain")

    return nc


def _pack_inputs(inputs, W, B_param):
    """Host-side shard + relayout. Returns list of 8 in_maps."""
    inputs = np.ascontiguousarray(inputs, dtype=np.float32)
    W = np.ascontiguousarray(W, dtype=np.float32)
    B_param = np.ascontiguousarray(B_param, dtype=np.float32)

    Wp = np.zeros((CH, NCP, DC, DIN), dtype=np.float32)
    Wp[:, :NC] = W
    Bp = np.zeros((NCP, DC), dtype=np.float32)
    Bp[:NC] = B_param

    # xt[(c,dc), dd, (b,rr)] = x[b, 4c+rr, 128dc+dd]
    x4 = inputs.reshape(B, CH, 4, 2, 128)           # b, c, rr, dc, dd
    xt = x4.transpose(1, 3, 4, 0, 2).reshape(CH * 2, 128, 128)
    bd4 = np.zeros((128, B), dtype=np.float32)
    bd4[np.arange(128), np.arange(128) // 4] = 1.0
    bd4t = bd4.T

    in_maps = []
    for core in range(NCORES):
        sl = slice(core * NSH, (core + 1) * NSH)
        Wc = Wp[:, sl]                               # c, n, k, d
        w5 = Wc.reshape(CH, NSH, DC, 2, 128)         # c n k dc dd
        wtc = w5.transpose(0, 3, 4, 1, 2).reshape(CH * 2, 128, NK)
        cstc = np.zeros((128, 544), dtype=np.float32)
        cstc[:, 0:B] = bd4
        cstc[0:B, B:B + 128] = bd4t
        cstc[0:B, B + 128:B + 128 + NK] = np.broadcast_to(
            Bp[sl].reshape(1, NK), (B, NK))
        xwc = np.concatenate([xt, wtc], axis=2)      # [64, 128, 512]
        in_maps.append(dict(xw=np.ascontiguousarray(xwc), cst=cstc))
    return in_maps


def _run(inputs, W, B_param, trace=False):
    from concourse.bass_utils import run_bass_kernel_spmd

    if "nc" not in _cache:
        _cache["nc"] = _build_nc()
    nc = _cache["nc"]
    in_maps = _pack_inputs(inputs, W, B_param)
    res = run_bass_kernel_spmd(nc, in_maps, core_ids=list(range(NCORES)),
                               trace=trace)
    outs = [r["out"].reshape(B, NSH, DC) for r in res.results]
    full = np.concatenate(outs, axis=1)[:, :NC, :]
    return np.ascontiguousarray(full.astype(np.float32)), res


def kernel(inputs, W, B_param):
    out, _ = _run(inputs, W, B_param, trace=False)
    return out



# revision 4
# speedup vs baseline: 2.1769x; 2.1769x over previous
"""Trainium2 Bass kernel for nn_CapsuleLayer (B=32, In=128, Din=256, ch=32, Nc=47, Dc=64).

Sharding: over the OUTPUT-CAPSULE axis Nc (47 -> pad 48 = 8 cores x 6 capsules).
Routing is fully independent per (batch, output-capsule), and W (94 MiB) is the
dominant HBM tensor -- Nc-sharding reads W exactly once total (12.6 MiB/core)
instead of replicating it 8x as batch-sharding would.

bf16 everywhere the 2e-2 tolerance allows (stream, inputs_hat, routing
elementwise); fp32 for PSUM accumulation, softmax normalization and squash.
Measured end-to-end numeric error of this pipeline ~5e-3.

Per-core layout: inputs_hat IH[p=(b,rr) 128 partitions, (c, k, n)] bf16 --
c-major so that
  * the a-step mul  TMP = IH * OUTr   broadcasts OUTr over the OUTER c axis
    (innermost reads stay step-1 -> DVE 2x bf16 mode),
  * the s-step mul  TS = IH * E       broadcasts E[p,c,n] over the MIDDLE k
    axis (innermost n runs step-1 -> 2x mode),
  * per-channel [128, (k,n)] blocks stay contiguous for PSUM copies and
    matmul rhs operands.
The k-reduction of the a-step runs as a pairwise TREE of 2x tensor_adds
(tensor_reduce is capped at 1x mode); the c+rr reductions of the s-step run
on the PE as PSUM-accumulated block-diagonal (BD4) matmuls.  sqrt in squash
is computed as exp(-0.5*ln) so ACT needs only the natural_log_exp table set
(no per-iteration table reloads).

Toolchain constraint: matmul (S3_LW) and DMA (DIRECT2D) instructions accept at
most ONE sync wait at codegen; DVE likewise.  Deps from the SAME engine merge
into one sem, so the kernel keeps every matmul's waits on a single engine;
const-DMA sems and psum-slot WAR ticks are pre-absorbed into the PE clock via
tiny dummy matmuls, and cross-engine (ACT<->DVE) handoffs via tiny copies.
"""

import numpy as np

B, IN, DIN = 32, 128, 256
CH, NC, DC = 32, 47, 64
NCP = 48          # padded Nc
NSH = 6           # capsules per core
NCORES = 8
NK = NSH * DC     # 384
EPS = 1e-7

_cache = {}


def _build_nc():
    import concourse.bass as bass
    import concourse.tile as tile
    from concourse import mybir
    from concourse.tile_rust import add_dep_helper

    f32 = mybir.dt.float32
    bf16 = mybir.dt.bfloat16
    nc = bass.Bass()

    # packed stream: per cd-chunk rows [xt(128) | wt(384)] = 512 bf16
    xw = nc.dram_tensor("xw", [CH * 2, 128, 512], bf16, kind="ExternalInput")
    # consts: bf16 bd4 [128,32]; fp32 [bd4 | bd4t(rows<32) | brep(rows<32)]
    cstb = nc.dram_tensor("cstb", [128, 32], bf16, kind="ExternalInput")
    cstf = nc.dram_tensor("cstf", [128, 544], f32, kind="ExternalInput")
    out_d = nc.dram_tensor("out", [B, NK], f32, kind="ExternalOutput")

    ADD = mybir.AluOpType.add
    MULT = mybir.AluOpType.mult
    AX = mybir.AxisListType.X
    AF = mybir.ActivationFunctionType

    with tile.TileContext(nc) as tc:
        with (
            tc.tile_pool(name="singles", bufs=1) as singles,
            tc.tile_pool(name="work", bufs=1) as work,
            tc.tile_pool(name="small", bufs=2) as small,
            tc.tile_pool(name="ps_ih", bufs=2, space="PSUM") as ps_ih,
            tc.tile_pool(name="ps_s1", bufs=1, space="PSUM") as ps_s1,
            tc.tile_pool(name="ps_rep", bufs=1, space="PSUM") as ps_rep,
            tc.tile_pool(name="ps_sm", bufs=1, space="PSUM") as ps_sm,
            tc.tile_pool(name="ps_s2", bufs=2, space="PSUM") as ps_s2,
        ):
            cstb_t = singles.tile([128, 32], bf16)
            cb_dma = nc.sync.dma_start(out=cstb_t[:], in_=cstb[:])
            cstf_t = singles.tile([128, 544], f32)
            cf_dma = nc.sync.dma_start(out=cstf_t[:], in_=cstf[:])
            bd4b_t = cstb_t[:, 0:32]            # bf16 [128, 32]
            bd4f_t = cstf_t[:, 0:B]             # fp32 [128, 32]
            bd4tf_t = cstf_t[0:B, B:B + 128]    # fp32 [32, 128]
            brep_t = cstf_t[0:B, B + 128:B + 128 + NK]  # fp32 [32, 384] (k,n)
            eps_t = singles.tile([B, 1], f32)
            nc.vector.memset(eps_t[:], EPS)
            # DVE is 1-wait-limited: pre-observe both const-DMA sems
            dve_scratch = singles.tile([B, 8], f32)
            nc.vector.tensor_copy(dve_scratch[:, 0:2], cstf_t[:B, :2])
            dve_scratch_b = singles.tile([B, 2], bf16)
            nc.vector.tensor_copy(dve_scratch_b[:], cstb_t[:B, :2])

            # IH[p, c, k, n] bf16 -- c-major inputs_hat
            IH = singles.tile([128, CH, DC, NSH], bf16)
            STREAM = singles.tile([128, CH * 2, 512], bf16)

            # Absorb the const-DMA sems into the PE clock (dummy matmul) so
            # real matmuls carry a single wait.
            pd = ps_sm.tile([2, 2], f32, tag="dummy")
            last_dummy = nc.tensor.matmul(
                pd[:], cstf_t[:2, :2], cstf_t[:2, :2], start=True, stop=True,
                skip_group_check=True,
            )
            dmy2 = nc.tensor.matmul(
                pd[:], cstb_t[:2, :2], cstb_t[:2, :2], start=True, stop=True,
                skip_group_check=True,
            )
            last_dummy = dmy2

            psum_s1 = ps_s1.tile([B, NK], f32)

            # ---------------- phase 1: inputs_hat + iter-1 s ----------------
            # 4 one-shot stream DMAs (8 channels each) so PE trails each chunk
            # while the total HWDGE instruction count stays within the queue
            # budget (the output DMA must not inherit a queue-WAR wait).
            NG = 4
            GCH = CH // NG          # channels per DMA group
            s_dmas = []
            for g in range(NG):
                s_dmas.append(nc.sync.dma_start(
                    out=STREAM[:, 2 * GCH * g:2 * GCH * (g + 1), :],
                    in_=xw[2 * GCH * g:2 * GCH * (g + 1)].rearrange(
                        "q d f -> d q f"),
                ))
            copy_insts = []
            for c in range(CH):
                if c >= 2:
                    # absorb the psum-slot WAR tick (copy of c-2, DVE/ACT sem)
                    dmy = nc.tensor.matmul(pd[:], cstb_t[:2, :2], cstb_t[:2, :2],
                                           start=True, stop=True,
                                           skip_group_check=True)
                    add_dep_helper(dmy.ins, copy_insts[c - 2].ins, sync=True,
                                   reason="absorb psum WAR tick on PE")
                    last_dummy = dmy
                psum_ih = ps_ih.tile([128, NK], f32, tag="ih")
                for dc in range(2):
                    cd = c * 2 + dc
                    mih = nc.tensor.matmul(
                        psum_ih[:], STREAM[:, cd, 0:128], STREAM[:, cd, 128:512],
                        start=(dc == 0), stop=(dc == 1),
                    )
                    if dc == 0:
                        add_dep_helper(mih.ins, last_dummy.ins, sync=False,
                                       reason="order dummy before matmul")
                # spread PSUM->SBUF (fp32 -> bf16) copies across DVE and ACT
                pv = psum_ih[:].rearrange("p (k n) -> p k n", n=NSH)
                if c % 2 == 0:
                    copy_insts.append(nc.vector.tensor_copy(IH[:, c], pv))
                else:
                    copy_insts.append(nc.scalar.copy(IH[:, c], pv))
                # iter-1 s accumulation: sum_rr over partitions via BD4, and
                # over channels via PSUM accumulation
                nc.tensor.matmul(
                    psum_s1[:], bd4b_t, IH[:, c].rearrange("p k n -> p (k n)"),
                    start=(c == 0), stop=(c == CH - 1),
                    skip_group_check=True,
                )

            _absn = [0]

            def absorb(eng, src_ap):
                """Tiny copy on `eng` reading src_ap: pre-observes the
                producer's sem so the next real op keeps a single wait."""
                _absn[0] += 1
                scr = small.tile([2, 2], f32, tag="abs%d" % _absn[0])
                if eng == "v":
                    return nc.vector.tensor_copy(scr[:], src_ap)
                return nc.scalar.copy(scr[:], src_ap)

            def squash(S, tag):
                """S: [B, (k,n)] fp32 sbuf tile -> OUT [B, (k,n)] fp32."""
                Ssq = work.tile([B, NK], f32, tag="Ssq")
                nc.vector.tensor_mul(Ssq[:], S[:], S[:])
                m2 = small.tile([B, NSH], f32, tag="m2")
                nc.vector.tensor_reduce(
                    m2[:], Ssq[:].rearrange("p (k n) -> p n k", n=NSH),
                    axis=AX, op=ADD,
                )
                d1 = small.tile([B, NSH], f32, tag="d1")
                nc.vector.tensor_scalar_add(d1[:], m2[:], 1.0)
                rd1 = small.tile([B, NSH], f32, tag="rd1")
                nc.vector.reciprocal(rd1[:], d1[:])
                absorb("s", m2[:2, :2])          # ACT clock <- m2 (DVE)
                # rsqrt(m2+eps) = exp(-0.5*ln(m2+eps)): keeps ACT on the
                # natural_log_exp table set (shared with softmax exp)
                lt = small.tile([B, NSH], f32, tag="lt")
                nc.scalar.activation(lt[:], m2[:], AF.Ln, bias=eps_t[:])
                rs = small.tile([B, NSH], f32, tag="rs")
                nc.scalar.activation(rs[:], lt[:], AF.Exp, scale=-0.5)
                absorb("v", rs[:2, :2])          # DVE clock <- rs (ACT)
                g0 = small.tile([B, NSH], f32, tag="g0")
                nc.vector.tensor_mul(g0[:], m2[:], rd1[:])
                g_ = small.tile([B, NSH], f32, tag="g")
                nc.vector.tensor_mul(g_[:], g0[:], rs[:])
                OUT = work.tile([B, NK], f32, tag="out")
                nc.vector.tensor_mul(
                    OUT[:].rearrange("p (k n) -> p k n", n=NSH),
                    S[:].rearrange("p (k n) -> p k n", n=NSH),
                    g_[:].rearrange("p (o n) -> p o n", o=1)
                       .broadcast_to([B, DC, NSH]),
                )
                return OUT

            def replicate(OUT, tag):
                """OUT [B, NK] fp32 -> OUTr [128, NK] bf16 (row b -> 4b..4b+3)."""
                pr = ps_rep.tile([128, NK], f32, tag="rep")
                nc.tensor.matmul(pr[:], bd4tf_t, OUT[:], start=True, stop=True)
                R = work.tile([128, NK], bf16, tag="OUTr")
                nc.vector.tensor_copy(R[:], pr[:])
                return R

            # ---------------- iter 1 ----------------
            S1 = work.tile([B, NK], f32, tag="S")
            nc.vector.scalar_tensor_tensor(
                out=S1[:], in0=psum_s1[:], scalar=1.0 / IN, in1=brep_t,
                op0=MULT, op1=ADD,
            )
            OUT1 = squash(S1, "1")
            OUTr = replicate(OUT1, "1")

            A2 = None
            for it in (2, 3):
                # ---- a-step: TMP = IH * OUTr (2x bf16); tree-reduce over k
                absorb("v", IH[:2, CH - 1, 0, 0:2])  # DVE clock <- last ACT copy
                TMP = work.tile([128, CH, DC, NSH], bf16, tag="TMP")
                nc.vector.tensor_mul(
                    TMP[:].rearrange("p c k n -> p c (k n)"),
                    IH[:].rearrange("p c k n -> p c (k n)"),
                    OUTr[:].rearrange("p (o kn) -> p o kn", o=1)
                          .broadcast_to([128, CH, NK]),
                )
                T = TMP
                kk = DC
                while kk > 2:
                    kk //= 2
                    Tn = work.tile([128, CH, kk, NSH], bf16, tag="T%d" % kk)
                    nc.vector.tensor_add(Tn[:], T[:, :, 0:kk, :],
                                         T[:, :, kk:2 * kk, :])
                    T = Tn
                A = work.tile([128, CH, NSH], f32, tag="A%d" % it)
                nc.vector.tensor_add(A[:], T[:, :, 0, :], T[:, :, 1, :])
                if A2 is None:
                    BL = A
                    A2 = A
                else:
                    BL = work.tile([128, CH, NSH], f32, tag="BL")
                    nc.vector.tensor_add(BL[:], A[:], A2[:])
                # ---- E = exp(BL) (bf16 out)
                absorb("s", BL[:2, 0, 0:2])       # ACT clock <- BL (DVE)
                E = work.tile([128, CH, NSH], bf16, tag="E")
                nc.scalar.activation(E[:], BL[:], AF.Exp)
                # ---- Z: sum_c on DVE, sum_rr on PE
                absorb("v", E[:2, 0, 0:2])        # DVE clock <- E (ACT)
                Zp = small.tile([128, NSH], f32, tag="Zp")
                nc.vector.tensor_reduce(
                    Zp[:], E[:].rearrange("p c n -> p n c"), axis=AX, op=ADD,
                )
                pz = ps_sm.tile([B, NSH], f32, tag="pz")
                nc.tensor.matmul(pz[:], bd4f_t, Zp[:], start=True, stop=True,
                                 skip_group_check=True)
                # ---- s-step: TS = E * IH (2x bf16, 4 c-chunks); c+rr sums on
                # PE as a 32-matmul PSUM accumulation group
                pS = ps_s2.tile([B, NK], f32, tag="pS")
                mm_last = None
                for ci in range(4):
                    cs = 8 * ci
                    TS = work.tile([128, 8, DC, NSH], bf16, tag="TS%d" % ci)
                    nc.vector.tensor_mul(
                        TS[:],
                        IH[:, cs:cs + 8],
                        E[:, cs:cs + 8, :].rearrange("p c (o n) -> p c o n", o=1)
                         .broadcast_to([128, 8, DC, NSH]),
                    )
                    for j in range(8):
                        c = cs + j
                        mm_last = nc.tensor.matmul(
                            pS[:], bd4b_t,
                            TS[:, j].rearrange("p k n -> p (k n)"),
                            start=(c == 0), stop=(c == CH - 1),
                            skip_group_check=True,
                        )
                Zs = small.tile([B, NSH], f32, tag="Zs")
                nc.vector.tensor_copy(Zs[:], pz[:])
                Rz = small.tile([B, NSH], f32, tag="Rz")
                nc.vector.reciprocal(Rz[:], Zs[:])
                absorb("v", pS[:2, :2])          # DVE clock <- pS (PE)
                Su = work.tile([B, NK], f32, tag="Su")
                nc.vector.tensor_mul(
                    Su[:].rearrange("p (k n) -> p k n", n=NSH),
                    pS[:].rearrange("p (k n) -> p k n", n=NSH),
                    Rz[:].rearrange("p (o n) -> p o n", o=1)
                       .broadcast_to([B, DC, NSH]),
                )
                S = work.tile([B, NK], f32, tag="S")
                nc.vector.tensor_add(S[:], Su[:], brep_t)
                OUT = squash(S, str(it))
                if it < 3:
                    OUTr = replicate(OUT, str(it))
                else:
                    o_dma = nc.sync.dma_start(out=out_d[:], in_=OUT[:])
                    # Pre-absorb every final sem into the SYNC engine so the
                    # Tile kernel-tail drain needs <=1 wait (codegen limit).
                    f_scr = small.tile([2, 4], f32, tag="fin")
                    f_act = nc.scalar.copy(f_scr[:, 0:2], OUT[:2, :2])
                    f_dve = nc.vector.tensor_copy(f_scr[:, 2:4], OUT[:2, :2])
                    for fin in ([cb_dma, cf_dma] + s_dmas +
                                [mm_last, f_act, f_dve, o_dma]):
                        fnop = nc.sync.nop()
                        add_dep_helper(fnop.ins, fin.ins, sync=True,
                                       reason="absorb final sem for tail drain")

    return nc


def _pack_inputs(inputs, W, B_param):
    """Host-side shard + relayout. Returns list of 8 in_maps."""
    import ml_dtypes

    bf = ml_dtypes.bfloat16
    inputs = np.ascontiguousarray(inputs, dtype=np.float32)
    W = np.ascontiguousarray(W, dtype=np.float32)
    B_param = np.ascontiguousarray(B_param, dtype=np.float32)

    Wp = np.zeros((CH, NCP, DC, DIN), dtype=np.float32)
    Wp[:, :NC] = W
    Bp = np.zeros((NCP, DC), dtype=np.float32)
    Bp[:NC] = B_param

    # xt[(c,dc), dd, (b,rr)] = x[b, 4c+rr, 128dc+dd]
    x4 = inputs.reshape(B, CH, 4, 2, 128)           # b, c, rr, dc, dd
    xt = x4.transpose(1, 3, 4, 0, 2).reshape(CH * 2, 128, 128).astype(bf)
    bd4 = np.zeros((128, B), dtype=np.float32)
    bd4[np.arange(128), np.arange(128) // 4] = 1.0
    bd4t = bd4.T

    in_maps = []
    for core in range(NCORES):
        sl = slice(core * NSH, (core + 1) * NSH)
        Wc = Wp[:, sl]                               # c, n, k, d
        w5 = Wc.reshape(CH, NSH, DC, 2, 128)         # c n k dc dd
        # columns in (k, n) order
        wtc = w5.transpose(0, 3, 4, 2, 1).reshape(CH * 2, 128, NK).astype(bf)
        cstfc = np.zeros((128, 544), dtype=np.float32)
        cstfc[:, 0:B] = bd4
        cstfc[0:B, B:B + 128] = bd4t
        # brep in (k, n) order
        cstfc[0:B, B + 128:B + 128 + NK] = np.broadcast_to(
            Bp[sl].T.reshape(1, NK), (B, NK))
        xwc = np.concatenate([xt, wtc], axis=2)      # [64, 128, 512] bf16
        in_maps.append(dict(xw=np.ascontiguousarray(xwc),
                            cstb=np.ascontiguousarray(bd4.astype(bf)),
                            cstf=cstfc))
    return in_maps


def _run(inputs, W, B_param, trace=False):
    from concourse.bass_utils import run_bass_kernel_spmd

    if "nc" not in _cache:
        _cache["nc"] = _build_nc()
    nc = _cache["nc"]
    in_maps = _pack_inputs(inputs, W, B_param)
    res = run_bass_kernel_spmd(nc, in_maps, core_ids=list(range(NCORES)),
                               trace=trace)
    # out cols are (k, n): reshape + transpose back to [B, n, k]
    outs = [r["out"].reshape(B, DC, NSH).transpose(0, 2, 1)
            for r in res.results]
    full = np.concatenate(outs, axis=1)[:, :NC, :]
    return np.ascontiguousarray(full.astype(np.float32)), res


def kernel(inputs, W, B_param):
    out, _ = _run(inputs, W, B_param, trace=False)
    return out
